# revision 39
# baseline (speedup 1.0000x reference)
"""EnsembleMLP kernel for 8x Trainium2 NeuronCores (Bass/Tile).

Computation (reference):
    hidden = tanh(x @ W_in.T)                            # (B, HID)
    ens_in = hidden[:, ids.ravel()].reshape(B, M, D)     # gather
    ens_pred = einsum('bmd,mod->bmo', ens_in, W_pred)    # per-member matvec (OUT=1)
    prediction = mean_m(ens_pred)                        # (B, 1)

Since OUT == 1, everything after `hidden` is linear in `hidden` and collapses
into a single vector:
    w_eff[h] = (1/M) * sum_{m,d : ids[m,d]==h} W_pred[m,0,d]
    prediction[b] = sum_h tanh(x @ W_in.T)[b,h] * w_eff[h]

The device kernel therefore computes tanh(x @ W_in.T) @ w_eff. The scatter-add
for w_eff (16K elements) runs on the host in fp64.

Sharding: data-parallel over batch. Each of the 8 cores gets B/8 = 1024 rows
of x; W_in and w_eff are replicated. No collectives needed.

Device layout per core (hidden kept transposed, [h_part, b_free]):
  - lhsT = W_in.T tiles [i=128, h=128], rhs = x.T tiles [i=128, b=512]
  - psum[h=128, b=512] accumulates over IN (8 k-tiles)
  - tanh on the scalar engine (PSUM -> SBUF)
  - weighted reduction over h via PE: acc[1, b=512] += w_eff[h_tile].T @ tanh_tile,
    accumulated across all 32 h-tiles in a persistent PSUM bank.
Matmuls run as float32r (reduced-precision fp32, full PE rate at N>=256).
"""

import sys

for _p in ("/opt/trn_rl_repo",):
    if _p not in sys.path:
        sys.path.insert(0, _p)

import numpy as np

B, IN, HID, M, D, OUT = 8192, 1024, 4096, 64, 256, 1
NCORES = 8
BSH = B // NCORES          # batch rows per core (1024)
NBC = BSH // 512           # 512-wide batch chunks per core (2)
NI = IN // 128             # k-tiles over IN (8)
NH = HID // 128            # h-tiles over HID (32)

# Set by test.py to profile; default off for grading speed.
TRACE = False

_CACHE = {}


def _build_nc():
    import concourse.bass as bass  # noqa: F401
    import concourse.mybir as mybir
    import concourse.tile as tile
    from concourse import bacc
    from contextlib import ExitStack

    F32 = mybir.dt.float32
    F32R = mybir.dt.float32r
    Tanh = mybir.ActivationFunctionType.Tanh

    Mult = mybir.AluOpType.mult
    Add = mybir.AluOpType.add

    # Bacc (not raw Bass): its compile() runs generate_event_semaphores,
    # which legalizes multi-sem waits down to the TRN2 1-wait-per-inst limit.
    nc = bacc.Bacc()
    xT = nc.dram_tensor("xT", [IN, BSH], F32R, kind="ExternalInput")
    # Host pre-arranges W into the exact SBUF tile layout wP[h, p, (i, hh)]
    # so every weight DMA is 128 contiguous 4KB rows. The natural
    # "(i p) h -> p i h" rearrange makes 512B descriptors, which caps the
    # W stream at ~155GB/s vs ~390GB/s for contiguous rows.
    wP = nc.dram_tensor("wP", [NH, 128, NI * 128], F32R, kind="ExternalInput")
    # Host pre-shuffles w_eff to [128, NH] (partition-major) so the DMA is
    # 128 contiguous rows, not a 4096-descriptor 4-byte gather.
    weff = nc.dram_tensor("weff", [128, NH], F32, kind="ExternalInput")
    out = nc.dram_tensor("out", [1, BSH], F32, kind="ExternalOutput")

    with tile.TileContext(nc) as tc, ExitStack() as ctx:
        xpool = ctx.enter_context(tc.tile_pool(name="x", bufs=1))
        wpool = ctx.enter_context(tc.tile_pool(name="w", bufs=1))
        cpool = ctx.enter_context(tc.tile_pool(name="c", bufs=1))
        tpool = ctx.enter_context(tc.tile_pool(name="t", bufs=6))
        apool = ctx.enter_context(tc.tile_pool(name="a", bufs=1))
        opool = ctx.enter_context(tc.tile_pool(name="o", bufs=1))
        pps = ctx.enter_context(tc.tile_pool(name="ps", bufs=6, space="PSUM"))
        pacc = ctx.enter_context(tc.tile_pool(name="acc", bufs=1, space="PSUM"))

        ones = cpool.tile([128, 1], F32)
        nc.gpsimd.memset(ones[:], 1.0)

        # Resident x.T tiles, split per batch chunk so the bc=0 sweep can
        # start after only half the activations have landed.
        xts = [[None] * NI for _ in range(NBC)]
        for bc in range(NBC):
            for i in range(NI):
                xt = xpool.tile([128, 512], F32R, tag=f"x{bc}_{i}", name=f"x{bc}_{i}")
                xts[bc][i] = xt

        weff_sb = cpool.tile([128, NH], F32)

        # All W_in.T blocks are persistent (128KB/partition) and written
        # exactly once, so their DMAs carry no WAR/WAW waits and each h
        # group's first matmul waits only on its own weight DMA.
        w3s = []
        for h in range(NH):
            w3 = wpool.tile([128, NI, 128], F32R, tag=f"w3_{h}", name=f"w3_{h}")
            w3s.append(w3)

        def _x_dma(bc, i):
            nc.sync.dma_start(
                xts[bc][i][:],
                xT[i * 128:(i + 1) * 128, bc * 512:(bc + 1) * 512],
            )

        def _w_dma(h):
            nc.sync.dma_start(w3s[h][:], wP[h])

        # The HW DGE serves one DMA at a time (striped over all engines at
        # full bandwidth) in round-robin lane order, so service order ==
        # emission order. Emit in exact consumption order: x00 + w3_0 to
        # start h=0 ASAP, rest of x_bc0 (h=0 consumes per-i), then the W
        # stream (delivery ~1.7us/tile vs ~1.8us/tile consumption), with the
        # bc=1 x tiles trickled into the late W stream where the accumulated
        # lead absorbs them.
        _x_dma(0, 0)
        _w_dma(0)
        nc.sync.dma_start(weff_sb[:], weff[:, :])
        for i in range(1, NI):
            _x_dma(0, i)
        for h in range(1, NH):
            _w_dma(h)
        # The W stream finishes ~70us; the bc=1 sweep doesn't read these
        # until ~83us, so placing them last costs nothing and never stalls
        # the W stream.
        for i in range(NI):
            _x_dma(1, i)

        # SBUF accumulators (one per batch chunk); weighted h-reduction runs
        # on the otherwise-idle DVE instead of stealing PE matmul slots.
        accs = [
            apool.tile([128, 512], F32, tag=f"acc{bc}", name=f"acc{bc}")
            for bc in range(NBC)
        ]

        for bc in range(NBC):
            for h in range(NH):
                ps = pps.tile([128, 512], F32)
                for i in range(NI):
                    nc.tensor.matmul(
                        ps[:],
                        w3s[h][:, i, :],
                        xts[bc][i][:],
                        start=(i == 0),
                        stop=(i == NI - 1),
                    )
                th = tpool.tile([128, 512], F32)
                nc.scalar.activation(th[:], ps[:], Tanh)
                w_col = weff_sb[:, h:h + 1]
                if h == 0:
                    nc.vector.tensor_scalar_mul(accs[bc][:], th[:], w_col)
                else:
                    nc.vector.scalar_tensor_tensor(
                        accs[bc][:], th[:], w_col, accs[bc][:], Mult, Add
                    )
            # Partition-dim reduction of acc via plain-fp32 ones-vector
            # matmuls (tail-only; the 4x fp32 rate is irrelevant). Split in
            # halves so the PSUM->SBUF copy overlaps the second matmul, and
            # store via the sync queue so GpSimd has nothing to drain on the
            # exit path.
            pso = pacc.tile([1, 512], F32, tag="po", name=f"po{bc}")
            osb = opool.tile([1, 512], F32, tag=f"o{bc}", name=f"o{bc}")
            nc.tensor.matmul(pso[:], ones[:], accs[bc][:], start=True, stop=True)
            nc.vector.tensor_copy(osb[:], pso[:])
            nc.sync.dma_start(out[0:1, bc * 512:(bc + 1) * 512], osb[:])

    nc.compile()
    return nc


def kernel(x, W_in, W_pred, ensemble_ids):
    from concourse.bass_utils import run_bass_kernel_spmd

    x = np.asarray(x, dtype=np.float32)
    W_in = np.asarray(W_in, dtype=np.float32)
    W_pred = np.asarray(W_pred, dtype=np.float32)
    ids = np.asarray(ensemble_ids)

    # Host: collapse gather + per-member matvec + mean into one HID-vector
    w_eff = np.zeros(HID, dtype=np.float64)
    np.add.at(w_eff, ids.ravel(), W_pred[:, 0, :].ravel().astype(np.float64))
    w_eff = (w_eff / M).astype(np.float32)
    # Device layout: [partition=h%128, free=h//128] (contiguous-row DMA)
    w_eff = np.ascontiguousarray(w_eff.reshape(NH, 128).T)

    xT = np.ascontiguousarray(x.T)        # [IN, B]
    # wP[h, p, i*128+hh] = W_in[h*128+hh, i*128+p]: the SBUF tile layout,
    # so each weight-block DMA is 128 contiguous 4KB rows.
    wP = np.ascontiguousarray(
        W_in.reshape(NH, 128, NI, 128).transpose(0, 3, 2, 1).reshape(NH, 128, NI * 128)
    )

    if "nc" not in _CACHE:
        _CACHE["nc"] = _build_nc()
    nc = _CACHE["nc"]

    in_maps = [
        {
            "xT": np.ascontiguousarray(xT[:, c * BSH:(c + 1) * BSH]),
            "wP": wP,
            "weff": w_eff,
        }
        for c in range(NCORES)
    ]
    res = run_bass_kernel_spmd(nc, in_maps, list(range(NCORES)), trace=TRACE)
    _CACHE["last_result"] = res

    pred = np.concatenate(
        [res.results[c]["out"].reshape(-1) for c in range(NCORES)]
    )
    return pred.reshape(B, OUT).astype(np.float32)


# revision 43
# speedup vs baseline: 1.0110x; 1.0110x over previous
"""EnsembleMLP kernel for 8x Trainium2 NeuronCores (Bass/Tile).

Computation (reference):
    hidden = tanh(x @ W_in.T)                            # (B, HID)
    ens_in = hidden[:, ids.ravel()].reshape(B, M, D)     # gather
    ens_pred = einsum('bmd,mod->bmo', ens_in, W_pred)    # per-member matvec (OUT=1)
    prediction = mean_m(ens_pred)                        # (B, 1)

Since OUT == 1, everything after `hidden` is linear in `hidden` and collapses
into a single vector:
    w_eff[h] = (1/M) * sum_{m,d : ids[m,d]==h} W_pred[m,0,d]
    prediction[b] = sum_h tanh(x @ W_in.T)[b,h] * w_eff[h]

The device kernel therefore computes tanh(x @ W_in.T) @ w_eff. The scatter-add
for w_eff (16K elements) runs on the host in fp64.

Sharding: data-parallel over batch. Each of the 8 cores gets B/8 = 1024 rows
of x; W_in and w_eff are replicated. No collectives needed.

Device layout per core (hidden kept transposed, [h_part, b_free]):
  - lhsT = W_in.T tiles [i=128, h=128], rhs = x.T tiles [i=128, b=512]
  - psum[h=128, b=512] accumulates over IN (8 k-tiles)
  - tanh on the scalar engine (PSUM -> SBUF)
  - weighted reduction over h via PE: acc[1, b=512] += w_eff[h_tile].T @ tanh_tile,
    accumulated across all 32 h-tiles in a persistent PSUM bank.
Matmuls run as float32r (reduced-precision fp32, full PE rate at N>=256).
"""

import sys

for _p in ("/opt/trn_rl_repo",):
    if _p not in sys.path:
        sys.path.insert(0, _p)

import numpy as np

B, IN, HID, M, D, OUT = 8192, 1024, 4096, 64, 256, 1
NCORES = 8
BSH = B // NCORES          # batch rows per core (1024)
NBC = BSH // 512           # 512-wide batch chunks per core (2)
NI = IN // 128             # k-tiles over IN (8)
NH = HID // 128            # h-tiles over HID (32)

# Set by test.py to profile; default off for grading speed.
TRACE = False

_CACHE = {}


def _build_nc():
    import concourse.bass as bass  # noqa: F401
    import concourse.mybir as mybir
    import concourse.tile as tile
    from concourse import bacc
    from contextlib import ExitStack

    F32 = mybir.dt.float32
    F32R = mybir.dt.float32r
    Tanh = mybir.ActivationFunctionType.Tanh

    Mult = mybir.AluOpType.mult
    Add = mybir.AluOpType.add

    # Bacc (not raw Bass): its compile() runs generate_event_semaphores,
    # which legalizes multi-sem waits down to the TRN2 1-wait-per-inst limit.
    nc = bacc.Bacc()
    xT = nc.dram_tensor("xT", [IN, BSH], F32R, kind="ExternalInput")
    # Host pre-arranges W into the exact SBUF tile layout wP[h, p, (i, hh)]
    # so every weight DMA is 128 contiguous 4KB rows. The natural
    # "(i p) h -> p i h" rearrange makes 512B descriptors, which caps the
    # W stream at ~155GB/s vs ~390GB/s for contiguous rows.
    wP = nc.dram_tensor("wP", [NH, 128, NI * 128], F32R, kind="ExternalInput")
    # Host pre-shuffles w_eff to [128, NH] (partition-major) so the DMA is
    # 128 contiguous rows, not a 4096-descriptor 4-byte gather; an extra
    # all-ones column rides along for the final partition reduction.
    weff = nc.dram_tensor("weff", [128, NH + 1], F32, kind="ExternalInput")
    out = nc.dram_tensor("out", [1, BSH], F32, kind="ExternalOutput")

    with tile.TileContext(nc) as tc, ExitStack() as ctx:
        xpool = ctx.enter_context(tc.tile_pool(name="x", bufs=1))
        wpool = ctx.enter_context(tc.tile_pool(name="w", bufs=1))
        cpool = ctx.enter_context(tc.tile_pool(name="c", bufs=1))
        tpool = ctx.enter_context(tc.tile_pool(name="t", bufs=6))
        apool = ctx.enter_context(tc.tile_pool(name="a", bufs=1))
        opool = ctx.enter_context(tc.tile_pool(name="o", bufs=1))
        pps = ctx.enter_context(tc.tile_pool(name="ps", bufs=6, space="PSUM"))
        pacc = ctx.enter_context(tc.tile_pool(name="acc", bufs=1, space="PSUM"))

        # Resident x.T tiles: one [128, NI, 512] tile per batch chunk loaded
        # by a single DMA (2KB descriptors; one ~0.6us sequencer dispatch
        # instead of eight, and h=0's matmuls pipeline as one warm burst).
        xts = [
            xpool.tile([128, NI, 512], F32R, tag=f"x{bc}", name=f"x{bc}")
            for bc in range(NBC)
        ]

        weff_sb = cpool.tile([128, NH + 1], F32)
        ones = weff_sb[:, NH:NH + 1]

        # All W_in.T blocks are persistent (128KB/partition) and written
        # exactly once, so their DMAs carry no WAR/WAW waits and each h
        # group's first matmul waits only on its own weight DMA.
        w3s = []
        for h in range(NH):
            w3 = wpool.tile([128, NI, 128], F32R, tag=f"w3_{h}", name=f"w3_{h}")
            w3s.append(w3)

        def _x_dma(bc):
            nc.sync.dma_start(
                xts[bc][:],
                xT[:, bc * 512:(bc + 1) * 512].rearrange("(i p) b -> p i b", p=128),
            )

        def _w_dma(h):
            nc.sync.dma_start(w3s[h][:], wP[h])

        # The HW DGE serves one DMA at a time (striped over all engines at
        # full bandwidth) in round-robin lane order, so service order ==
        # emission order. Emit in exact consumption order: w3_0 + x_bc0 to
        # start h=0 ASAP, then the W stream (delivery ~1.4us/tile vs
        # ~1.8us/tile consumption), then the bc=1 x block (not read until
        # ~halfway through the kernel).
        _w_dma(0)
        nc.sync.dma_start(weff_sb[:], weff[:, :])
        _x_dma(0)
        for h in range(1, NH):
            _w_dma(h)
        for bc in range(1, NBC):
            _x_dma(bc)

        # SBUF accumulators (one per batch chunk); weighted h-reduction runs
        # on the otherwise-idle DVE instead of stealing PE matmul slots.
        accs = [
            apool.tile([128, 512], F32, tag=f"acc{bc}", name=f"acc{bc}")
            for bc in range(NBC)
        ]

        for bc in range(NBC):
            for h in range(NH):
                ps = pps.tile([128, 512], F32)
                for i in range(NI):
                    nc.tensor.matmul(
                        ps[:],
                        w3s[h][:, i, :],
                        xts[bc][:, i, :],
                        start=(i == 0),
                        stop=(i == NI - 1),
                    )
                th = tpool.tile([128, 512], F32)
                nc.scalar.activation(th[:], ps[:], Tanh)
                w_col = weff_sb[:, h:h + 1]
                if h == 0:
                    nc.vector.tensor_scalar_mul(accs[bc][:], th[:], w_col)
                else:
                    nc.vector.scalar_tensor_tensor(
                        accs[bc][:], th[:], w_col, accs[bc][:], Mult, Add
                    )
            # Partition-dim reduction of acc via plain-fp32 ones-vector
            # matmuls (tail-only; the 4x fp32 rate is irrelevant). Split in
            # halves so the PSUM->SBUF copy overlaps the second matmul, and
            # store via the sync queue so GpSimd has nothing to drain on the
            # exit path.
            pso = pacc.tile([1, 512], F32, tag="po", name=f"po{bc}")
            osb = opool.tile([1, 512], F32, tag=f"o{bc}", name=f"o{bc}")
            nc.tensor.matmul(pso[:], ones[:], accs[bc][:], start=True, stop=True)
            nc.vector.tensor_copy(osb[:], pso[:])
            nc.sync.dma_start(out[0:1, bc * 512:(bc + 1) * 512], osb[:])

    nc.compile()
    return nc


def kernel(x, W_in, W_pred, ensemble_ids):
    from concourse.bass_utils import run_bass_kernel_spmd

    x = np.asarray(x, dtype=np.float32)
    W_in = np.asarray(W_in, dtype=np.float32)
    W_pred = np.asarray(W_pred, dtype=np.float32)
    ids = np.asarray(ensemble_ids)

    # Host: collapse gather + per-member matvec + mean into one HID-vector
    w_eff = np.zeros(HID, dtype=np.float64)
    np.add.at(w_eff, ids.ravel(), W_pred[:, 0, :].ravel().astype(np.float64))
    w_eff = (w_eff / M).astype(np.float32)
    # Device layout: [partition=h%128, free=h//128] (contiguous-row DMA),
    # plus an all-ones column used by the final partition reduction.
    w_eff = np.ascontiguousarray(
        np.concatenate([w_eff.reshape(NH, 128).T, np.ones((128, 1), np.float32)], axis=1)
    )

    xT = np.ascontiguousarray(x.T)        # [IN, B]
    # wP[h, p, i*128+hh] = W_in[h*128+hh, i*128+p]: the SBUF tile layout,
    # so each weight-block DMA is 128 contiguous 4KB rows.
    wP = np.ascontiguousarray(
        W_in.reshape(NH, 128, NI, 128).transpose(0, 3, 2, 1).reshape(NH, 128, NI * 128)
    )

    if "nc" not in _CACHE:
        _CACHE["nc"] = _build_nc()
    nc = _CACHE["nc"]

    in_maps = [
        {
            "xT": np.ascontiguousarray(xT[:, c * BSH:(c + 1) * BSH]),
            "wP": wP,
            "weff": w_eff,
        }
        for c in range(NCORES)
    ]
    res = run_bass_kernel_spmd(nc, in_maps, list(range(NCORES)), trace=TRACE)
    _CACHE["last_result"] = res

    pred = np.concatenate(
        [res.results[c]["out"].reshape(-1) for c in range(NCORES)]
    )
    return pred.reshape(B, OUT).astype(np.float32)


# revision 47
# speedup vs baseline: 1.0176x; 1.0065x over previous
"""EnsembleMLP kernel for 8x Trainium2 NeuronCores (Bass/Tile).

Computation (reference):
    hidden = tanh(x @ W_in.T)                            # (B, HID)
    ens_in = hidden[:, ids.ravel()].reshape(B, M, D)     # gather
    ens_pred = einsum('bmd,mod->bmo', ens_in, W_pred)    # per-member matvec (OUT=1)
    prediction = mean_m(ens_pred)                        # (B, 1)

Since OUT == 1, everything after `hidden` is linear in `hidden` and collapses
into a single vector:
    w_eff[h] = (1/M) * sum_{m,d : ids[m,d]==h} W_pred[m,0,d]
    prediction[b] = sum_h tanh(x @ W_in.T)[b,h] * w_eff[h]

The device kernel therefore computes tanh(x @ W_in.T) @ w_eff. The scatter-add
for w_eff (16K elements) runs on the host in fp64.

Sharding: data-parallel over batch. Each of the 8 cores gets B/8 = 1024 rows
of x; W_in and w_eff are replicated. No collectives needed.

Device layout per core (hidden kept transposed, [h_part, b_free]):
  - lhsT = W_in.T tiles [i=128, h=128], rhs = x.T tiles [i=128, b=512]
  - psum[h=128, b=512] accumulates over IN (8 k-tiles)
  - tanh on the scalar engine (PSUM -> SBUF)
  - weighted reduction over h via PE: acc[1, b=512] += w_eff[h_tile].T @ tanh_tile,
    accumulated across all 32 h-tiles in a persistent PSUM bank.
Matmuls run as float32r (reduced-precision fp32, full PE rate at N>=256).
"""

import sys

for _p in ("/opt/trn_rl_repo",):
    if _p not in sys.path:
        sys.path.insert(0, _p)

import numpy as np

B, IN, HID, M, D, OUT = 8192, 1024, 4096, 64, 256, 1
NCORES = 8
BSH = B // NCORES          # batch rows per core (1024)
NBC = BSH // 512           # 512-wide batch chunks per core (2)
NI = IN // 128             # k-tiles over IN (8)
NH = HID // 128            # h-tiles over HID (32)

# Set by test.py to profile; default off for grading speed.
TRACE = False

_CACHE = {}


def _build_nc():
    import concourse.bass as bass  # noqa: F401
    import concourse.mybir as mybir
    import concourse.tile as tile
    from concourse import bacc
    from contextlib import ExitStack

    F32 = mybir.dt.float32
    F32R = mybir.dt.float32r
    Tanh = mybir.ActivationFunctionType.Tanh

    Mult = mybir.AluOpType.mult
    Add = mybir.AluOpType.add

    # Bacc (not raw Bass): its compile() runs generate_event_semaphores,
    # which legalizes multi-sem waits down to the TRN2 1-wait-per-inst limit.
    nc = bacc.Bacc()
    # Host pre-arranges x into per-batch-chunk SBUF tile layout
    # xP[bc, p, i*512+b] so each chunk's DMA is 128 contiguous 16KB rows
    # (~390GB/s vs ~215GB/s for the 2KB-descriptor on-the-fly rearrange).
    xP = nc.dram_tensor("xP", [NBC, 128, NI * 512], F32R, kind="ExternalInput")
    # Host pre-arranges W into the exact SBUF tile layout wP[h, p, (i, hh)]
    # so every weight DMA is 128 contiguous 4KB rows. The natural
    # "(i p) h -> p i h" rearrange makes 512B descriptors, which caps the
    # W stream at ~155GB/s vs ~390GB/s for contiguous rows.
    wP = nc.dram_tensor("wP", [NH, 128, NI * 128], F32R, kind="ExternalInput")
    # Host pre-shuffles w_eff to [128, NH] (partition-major) so the DMA is
    # 128 contiguous rows, not a 4096-descriptor 4-byte gather; an extra
    # all-ones column rides along for the final partition reduction.
    weff = nc.dram_tensor("weff", [128, NH + 1], F32, kind="ExternalInput")
    out = nc.dram_tensor("out", [1, BSH], F32, kind="ExternalOutput")

    with tile.TileContext(nc) as tc, ExitStack() as ctx:
        xpool = ctx.enter_context(tc.tile_pool(name="x", bufs=1))
        wpool = ctx.enter_context(tc.tile_pool(name="w", bufs=1))
        cpool = ctx.enter_context(tc.tile_pool(name="c", bufs=1))
        tpool = ctx.enter_context(tc.tile_pool(name="t", bufs=6))
        apool = ctx.enter_context(tc.tile_pool(name="a", bufs=1))
        opool = ctx.enter_context(tc.tile_pool(name="o", bufs=1))
        pps = ctx.enter_context(tc.tile_pool(name="ps", bufs=6, space="PSUM"))
        pacc = ctx.enter_context(tc.tile_pool(name="acc", bufs=1, space="PSUM"))

        # Resident x.T tiles: one [128, NI, 512] tile per batch chunk loaded
        # by a single DMA (2KB descriptors; one ~0.6us sequencer dispatch
        # instead of eight, and h=0's matmuls pipeline as one warm burst).
        xts = [
            xpool.tile([128, NI, 512], F32R, tag=f"x{bc}", name=f"x{bc}")
            for bc in range(NBC)
        ]

        weff_sb = cpool.tile([128, NH + 1], F32)
        ones = weff_sb[:, NH:NH + 1]

        # All W_in.T blocks are persistent (128KB/partition) and written
        # exactly once, so their DMAs carry no WAR/WAW waits and each h
        # group's first matmul waits only on its own weight DMA.
        w3s = []
        for h in range(NH):
            w3 = wpool.tile([128, NI, 128], F32R, tag=f"w3_{h}", name=f"w3_{h}")
            w3s.append(w3)

        def _x_dma(bc):
            nc.sync.dma_start(xts[bc][:], xP[bc])

        def _w_dma(h):
            nc.sync.dma_start(w3s[h][:], wP[h])

        # The HW DGE serves one DMA at a time (striped over all engines at
        # full bandwidth) in round-robin lane order, so service order ==
        # emission order. Emit in exact consumption order: w3_0 + x_bc0 to
        # start h=0 ASAP, then the W stream (delivery ~1.4us/tile vs
        # ~1.8us/tile consumption), then the bc=1 x block (not read until
        # ~halfway through the kernel).
        _w_dma(0)
        nc.sync.dma_start(weff_sb[:], weff[:, :])
        _x_dma(0)
        for h in range(1, NH):
            _w_dma(h)
        for bc in range(1, NBC):
            _x_dma(bc)

        # SBUF accumulators (one per batch chunk); weighted h-reduction runs
        # on the otherwise-idle DVE instead of stealing PE matmul slots.
        accs = [
            apool.tile([128, 512], F32, tag=f"acc{bc}", name=f"acc{bc}")
            for bc in range(NBC)
        ]

        for bc in range(NBC):
            for h in range(NH):
                ps = pps.tile([128, 512], F32)
                for i in range(NI):
                    nc.tensor.matmul(
                        ps[:],
                        w3s[h][:, i, :],
                        xts[bc][:, i, :],
                        start=(i == 0),
                        stop=(i == NI - 1),
                    )
                th = tpool.tile([128, 512], F32)
                nc.scalar.activation(th[:], ps[:], Tanh)
                w_col = weff_sb[:, h:h + 1]
                if h == 0:
                    nc.vector.tensor_scalar_mul(accs[bc][:], th[:], w_col)
                else:
                    nc.vector.scalar_tensor_tensor(
                        accs[bc][:], th[:], w_col, accs[bc][:], Mult, Add
                    )
            # Partition-dim reduction of acc via plain-fp32 ones-vector
            # matmuls (tail-only; the 4x fp32 rate is irrelevant). Split in
            # halves so the PSUM->SBUF copy overlaps the second matmul, and
            # store via the sync queue so GpSimd has nothing to drain on the
            # exit path.
            pso = pacc.tile([1, 512], F32, tag=f"po{bc}", name=f"po{bc}")
            osb = opool.tile([1, 512], F32, tag=f"o{bc}", name=f"o{bc}")
            nc.tensor.matmul(pso[:], ones[:], accs[bc][:], start=True, stop=True)
            nc.vector.tensor_copy(osb[:], pso[:])
            nc.sync.dma_start(out[0:1, bc * 512:(bc + 1) * 512], osb[:])

    nc.compile()
    return nc


def kernel(x, W_in, W_pred, ensemble_ids):
    from concourse.bass_utils import run_bass_kernel_spmd

    x = np.asarray(x, dtype=np.float32)
    W_in = np.asarray(W_in, dtype=np.float32)
    W_pred = np.asarray(W_pred, dtype=np.float32)
    ids = np.asarray(ensemble_ids)

    # Host: collapse gather + per-member matvec + mean into one HID-vector
    w_eff = np.zeros(HID, dtype=np.float64)
    np.add.at(w_eff, ids.ravel(), W_pred[:, 0, :].ravel().astype(np.float64))
    w_eff = (w_eff / M).astype(np.float32)
    # Device layout: [partition=h%128, free=h//128] (contiguous-row DMA),
    # plus an all-ones column used by the final partition reduction.
    w_eff = np.ascontiguousarray(
        np.concatenate([w_eff.reshape(NH, 128).T, np.ones((128, 1), np.float32)], axis=1)
    )

    # wP[h, p, i*128+hh] = W_in[h*128+hh, i*128+p]: the SBUF tile layout,
    # so each weight-block DMA is 128 contiguous 4KB rows.
    wP = np.ascontiguousarray(
        W_in.reshape(NH, 128, NI, 128).transpose(0, 3, 2, 1).reshape(NH, 128, NI * 128)
    )

    def _xP(c):
        # xP[bc, p, i*512+b] = x[c*BSH + bc*512 + b, i*128 + p]
        xs = x[c * BSH:(c + 1) * BSH, :]
        return np.ascontiguousarray(
            xs.reshape(NBC, 512, NI, 128).transpose(0, 3, 2, 1).reshape(NBC, 128, NI * 512)
        )

    if "nc" not in _CACHE:
        _CACHE["nc"] = _build_nc()
    nc = _CACHE["nc"]

    in_maps = [
        {"xP": _xP(c), "wP": wP, "weff": w_eff}
        for c in range(NCORES)
    ]
    res = run_bass_kernel_spmd(nc, in_maps, list(range(NCORES)), trace=TRACE)
    _CACHE["last_result"] = res

    pred = np.concatenate(
        [res.results[c]["out"].reshape(-1) for c in range(NCORES)]
    )
    return pred.reshape(B, OUT).astype(np.float32)
